# revision 4
# baseline (speedup 1.0000x reference)
"""Trainium2 Bass kernel for nn_DGG_LearnableK_Small (byte- and
engine-minimized single-stream design).

The reference collapses analytically:
  - softmax over a size-1 axis == 1, so log_p == 0 and edge_prob == 1/N
    exactly; stable argsort of a constant row is the identity permutation,
    so idx[b,i,j] = j and the scatter/gather permutations are identity.
  - adj[b,i,j] = sigmoid(2 - 7j + 7*(k[b,i]-1)) where
    k = (relu(x @ W_mu1 + b_mu1) @ W_mu2 + b_mu2) @ W_kp + b_kp.

Trace-driven design (one HWDGE ring streams ~385-420 GB/s; everything else
must hide under it):
  - idx streams as int16 (values <= 2047, lossless); host widens to int32
    during unshard.  4.19 MB/core instead of 8.39.
  - the full-width [128,2048] int16 iota is built fast: gpsimd iota seeds
    512 columns (~0.9us), two DVE adds extend to 2048 (iota itself runs at
    only ~71 elem/ns).  ONE broadcast-source dma_start (stride-0 rc axis)
    then writes all 8 row-chunks: 4 KB/row descriptors, single trigger.
  - the packed input is bf16 (x.T, W1, wv7, b1, ones) -> 492 KB, loaded on
    the ACT ring so the SP ring carries only the idx stream.  cke rides as
    a bf16 hi/lo pair summed to f32 on DVE.
  - b_mu1 is folded in via a [1,128]-ones x [1,256]-b1 outer-product
    matmul accumulated into the same PSUM group as x @ W1.
  - per chunk the whole relu/dot-product is ONE DVE scalar_tensor_tensor:
    hm = max(g,0) * wv7 with accum_out = shift (v2's ACT Copy+accum chain
    made ACT the 12us critical engine); ACT keeps only the 8 sigmoids.
  - adj streams as bf16 (~0.4% rounding, gate 2e-2); host widens to f32.

Sigmoid underflows to exactly 0.0 for j >= ~16 at any plausible shift;
run_bass_via_pjrt zero-fills output buffers, so adj only writes its first
CUT=128 columns (16x margin) and columns 128..2047 stay exactly 0.
"""

import os

import numpy as np

B, N, D, L = 4, 2048, 128, 256
NCORES = 8
ROWS = B * N          # 8192
RPC = ROWS // NCORES  # 1024 rows per core
P = 128
RCHUNKS = RPC // P    # 8
SEED = 512            # iota seed width; DVE doubles it twice to N
INTERVAL = 7.0
HS_START = 2.0
CUT = 32              # adj columns actually written (rest stay 0); the
                      # reference's last nonzero column is 13 and the
                      # theoretical max at k<=1.54 is 15, so 2x margin.
# xpb (bf16) layout: [xt | w1 | wv7 | b1row | ones | cke_hi cke_lo]
O_W1 = RPC            # 1024
O_WV = O_W1 + L       # 1280
O_B1 = O_WV + L       # 1536 (partition 0 only)
O_ONE = O_B1 + L      # 1792 (partition 0 only)
O_CKE = O_ONE + P     # 1920
XB = 2048             # padded past 1922 so xp rows are exactly 4 KB; the
                      # unpadded 3844 B descriptors streamed at only
                      # ~220-260 GB/s

_CACHE = {}

# Results of the last device run (exec time etc.) for the local test harness.
LAST_RESULTS = None


def _build_nc():
    import concourse.bacc as bacc
    import concourse.mybir as mybir
    from concourse.tile import TileContext

    f32 = mybir.dt.float32
    bf16 = mybir.dt.bfloat16
    i16 = mybir.dt.int16
    AF = mybir.ActivationFunctionType
    OP = mybir.AluOpType

    nc = bacc.Bacc(None, target_bir_lowering=False, debug=False)
    xpb = nc.declare_dram_parameter("xpb", [P, XB], bf16, isOutput=False)
    # adj leaves the device in fk's native [P, RCHUNKS*CUT] layout (contiguous
    # 2 KB rows -> 128 descriptors instead of 1024 x 256 B); the host
    # de-swizzles to [RPC, CUT] during unshard.
    adj = nc.declare_dram_parameter("adj", [P, RCHUNKS * CUT], bf16,
                                    isOutput=True)
    idx = nc.declare_dram_parameter("idx", [RPC, N], i16, isOutput=True)

    with TileContext(nc) as tc:
        with (
            tc.tile_pool(name="const", bufs=1) as cpool,
            tc.tile_pool(name="hps", bufs=4, space="PSUM") as hpool,
            tc.tile_pool(name="wk", bufs=4) as wpool,
        ):
            xp_sb = cpool.tile([P, XB], bf16, tag="xpb")
            # xp loads FIRST on the SP ring: it streams 8.3-9.7us, inside the
            # window where the ring would otherwise idle waiting for the iota
            # chain (idx-A's source is ready at ~8.6us), so the idx stream
            # starts the moment xp's last descriptor drains instead of paying
            # its own first-byte latency.
            nc.sync.dma_start(out=xp_sb, in_=xpb[:])

            # Full-width int16 iota: gpsimd seeds 512 cols, DVE extends.
            iot = cpool.tile([P, N], i16, tag="iot")
            nc.gpsimd.iota(iot[:, 0:SEED], pattern=[[1, SEED]], base=0,
                           channel_multiplier=0)
            nc.vector.tensor_scalar_add(iot[:, SEED:2 * SEED],
                                        iot[:, 0:SEED], SEED)
            nc.vector.tensor_scalar_add(iot[:, 2 * SEED:N],
                                        iot[:, 0:2 * SEED], 2 * SEED)
            # Broadcast-source DMAs (stride-0 rc axis) write every row-chunk
            # of idx.  Split at the seed boundary so the first megabyte's
            # descriptors enter the ring ~1us before the DVE extends finish.
            for c0, c1 in ((0, SEED), (SEED, N)):
                src = iot[:, c0:c1].rearrange("p (one c) -> p one c", one=1)
                src = src.to_broadcast((P, RCHUNKS, c1 - c0))
                nc.sync.dma_start(
                    out=idx[:, c0:c1].rearrange("(rc p) c -> p rc c", p=P),
                    in_=src,
                )

            iof_sb = cpool.tile([P, CUT], f32, tag="iof")
            nc.gpsimd.iota(iof_sb, pattern=[[1, CUT]], base=0,
                           channel_multiplier=0,
                           allow_small_or_imprecise_dtypes=True)

            # cke back to f32 from the bf16 hi/lo pair.
            ckef = cpool.tile([P, 1], f32, tag="ckef")
            nc.vector.tensor_tensor(
                ckef, xp_sb[:, O_CKE:O_CKE + 1], xp_sb[:, O_CKE + 1:O_CKE + 2],
                OP.add)

            # iof2[p, j] = -7*j + cke
            iof2 = cpool.tile([P, CUT], f32, tag="iof2")
            nc.vector.tensor_scalar(iof2, iof_sb, -INTERVAL, ckef,
                                    OP.mult, OP.add)

            w1_ap = xp_sb[:, O_W1:O_W1 + L]
            wv_ap = xp_sb[:, O_WV:O_WV + L]
            b1_ap = xp_sb[0:1, O_B1:O_B1 + L]
            one_ap = xp_sb[0:1, O_ONE:O_ONE + P]

            shift_all = cpool.tile([P, RCHUNKS], f32, tag="shift")
            fk = cpool.tile([P, RCHUNKS * CUT], bf16, tag="fk")
            for rc in range(RCHUNKS):
                g_ps = hpool.tile([P, L], f32, tag="hps")
                # b1 enters as a ones-column outer product, then x @ W1
                # accumulates on top: g = x @ W1 + b1 straight in PSUM.
                nc.tensor.matmul(g_ps, lhsT=one_ap, rhs=b1_ap,
                                 start=True, stop=False)
                nc.tensor.matmul(g_ps, lhsT=xp_sb[:, rc * P:(rc + 1) * P],
                                 rhs=w1_ap, start=False, stop=True)
                # hm = max(g,0) * wv7; shift = sum(hm) — one DVE op.
                hm = wpool.tile([P, L], bf16, tag="hm")
                nc.vector.scalar_tensor_tensor(
                    hm, g_ps, 0.0, wv_ap, op0=OP.max, op1=OP.mult,
                    accum_out=shift_all[:, rc:rc + 1])
                nc.scalar.activation(
                    fk[:, rc * CUT:(rc + 1) * CUT],
                    iof2,
                    AF.Sigmoid,
                    bias=shift_all[:, rc:rc + 1],
                    scale=1.0,
                )
            # adj rides the SAME SP ring as idx, emitted after it: same-ring
            # FIFO appends its 64 KB cleanly after the idx stream's last byte
            # (~0.2us).  Any concurrent path (SWDGE or the other HWDGE ring)
            # halves aggregate throughput while both are active — measured as
            # a 1.5-2.5us stretch of the idx tail.
            nc.sync.dma_start(out=adj[:], in_=fk)

    nc.compile()
    return nc


def _pack_inputs(inputs):
    import concourse.mybir as mybir

    BF16 = mybir.dt.np(mybir.dt.bfloat16)

    x = np.ascontiguousarray(np.asarray(inputs["x"], dtype=np.float32))
    W1 = np.asarray(inputs["W_mu1"], dtype=np.float32)
    b1v = np.asarray(inputs["b_mu1"], dtype=np.float32)
    W2 = np.asarray(inputs["W_mu2"], dtype=np.float32)
    b2v = np.asarray(inputs["b_mu2"], dtype=np.float32)
    Wkp = np.asarray(inputs["W_kp"], dtype=np.float32)
    bkp = np.asarray(inputs["b_kp"], dtype=np.float32)

    # Host-side folding of the linear tail (replicated across cores).
    wv7 = (W2 @ (np.float32(INTERVAL) * Wkp[:, 0])).astype(np.float32)
    cke = np.float32(HS_START) + np.float32(INTERVAL) * np.float32(
        b2v @ Wkp[:, 0] + bkp[0])
    chi = np.float32(cke).astype(BF16)
    clo = (np.float32(cke) - np.float32(chi)).astype(BF16)

    x_flat = x.reshape(ROWS, D)
    base = np.zeros((P, XB), dtype=BF16)
    base[:, O_W1:O_W1 + L] = W1.astype(BF16)
    base[:, O_WV:O_WV + L] = wv7[None, :].astype(BF16)
    base[0, O_B1:O_B1 + L] = b1v.astype(BF16)
    base[0, O_ONE:O_ONE + P] = 1.0
    base[:, O_CKE] = chi
    base[:, O_CKE + 1] = clo

    in_maps = []
    for c in range(NCORES):
        xpack = base.copy()
        xpack[:, 0:RPC] = x_flat[c * RPC:(c + 1) * RPC].T.astype(BF16)
        in_maps.append({"xpb": xpack})
    return in_maps


def kernel(**inputs):
    global LAST_RESULTS
    from concourse.bass_utils import run_bass_kernel_spmd

    if "nc" not in _CACHE:
        _CACHE["nc"] = _build_nc()
    nc = _CACHE["nc"]

    in_maps = _pack_inputs(inputs)

    try:
        res = run_bass_kernel_spmd(nc, in_maps, list(range(NCORES)))
    except ModuleNotFoundError:
        # BASS_TRACE was set in an environment without the axon NTFF hook
        # module; retry with tracing forced off.
        os.environ["BASS_NEVER_TRACE"] = "1"
        res = run_bass_kernel_spmd(nc, in_maps, list(range(NCORES)))
    LAST_RESULTS = res

    adj_full = np.zeros((ROWS, N), dtype=np.float32)
    idx_full = np.empty((ROWS, N), dtype=np.int32)
    for c in range(NCORES):
        a = res.results[c]["adj"].astype(np.float32)
        adj_full[c * RPC:(c + 1) * RPC, 0:CUT] = (
            a.reshape(P, RCHUNKS, CUT).transpose(1, 0, 2).reshape(RPC, CUT))
        idx_full[c * RPC:(c + 1) * RPC] = res.results[c]["idx"].astype(
            np.int32)

    return adj_full.reshape(B, N, N), idx_full.reshape(B, N, N)
